# revision 11
# baseline (speedup 1.0000x reference)
"""Trainium2 Bass kernel for FISTA sparse coding (nn_FISTA_7550552506950).

Strategy (data-parallel over batch, 8 cores x 128 rows), v2:
- State z kept TRANSPOSED [F=4096, B=128] on-chip as fp32 (streamed to the PE
  as float32r), split into 32 f-chunks of [128, 256] (real|imag halves).
- True-residual gradient: instead of streaming O(1)-magnitude quadrant
  combinations through the gradient matmuls (which forces >=12-bit weights and
  2 matmuls/chunk), the tiny complex residual r = Dw - x is materialized once
  per iteration [64, B] and streamed as the fp16 concatenation
  [r_re|r_im ; r_im|-r_re], so ONE fp16 matmul per chunk produces both the
  real and imag gradient halves. fp16 is safe there because r is small and
  the cancellation already happened in fp32.
- A-chain (P1 = D @ z): one f32r matmul per chunk streaming z directly; the
  momentum combine A(w) = a*A(z) + b*A(z_old) happens on the tiny P1 tile.
- Momentum a-term (a*z) enters PSUM via f32r scaled-identity matmuls
  (~13-bit z stream, verified acceptable); b-term (b*z_old) is added by a
  fp32-exact DVE scalar_tensor_tensor, so the iterate keeps full precision.
  a = fp16-rounded (exact), b = 1-a fp32 => coefficient rounding cancels.
- Soft-threshold: m2 = ur^2+ui^2 (split DVE/GPS), rsqrt (ACT raw),
  s = relu(1-thr*rsq) (ACT), z = u*s (DVE). Final |z| = sqrt(m2)*s as fp16.
- Global max normalization on host during the gather (tiny).
"""

import numpy as np
from contextlib import ExitStack

import concourse.bass as bass
import concourse.mybir as mybir
import concourse.tile as tile
from concourse import bacc
from concourse.bass_utils import run_bass_kernel_spmd

F32 = mybir.dt.float32
F32R = mybir.dt.float32r
BF16 = mybir.dt.bfloat16
FP16 = mybir.dt.float16
ALU = mybir.AluOpType
ACTF = mybir.ActivationFunctionType

P = 128          # partitions / f-chunk size
F = 4096         # dictionary size
T = 64           # signal dim
NCH = F // P     # 32 chunks
B = 128          # batch rows per core
NCORES = 8
MAX_ITER = 25
STEP = np.float32(1.0 / F)
THR = np.float32(0.5) * STEP
GRP = 4          # chunks per elementwise group
NGRP = NCH // GRP
ALAG = 2         # groups of delay before A-chain streams fresh z


def _activation_raw(nc, out, in_, func, bias, scale=1.0):
    """nc.scalar.activation minus the Rsqrt accuracy guard.

    Safe here: rsqrt feeds only the soft-threshold scale, where its error is
    attenuated by thr/mag (absolute z error <= eps * thr ~ 1e-6); the final
    output magnitude uses the accurate Sqrt path instead.
    """
    inputs = [nc.scalar.lower_ap(in_)]
    for arg in (bias, scale, 0.0):
        if isinstance(arg, float):
            inputs.append(mybir.ImmediateValue(dtype=F32, value=arg))
        else:
            inputs.append(nc.scalar.lower_ap(arg))
    return nc.scalar.add_instruction(
        mybir.InstActivation(
            name=nc.get_next_instruction_name(),
            func=func,
            ins=inputs,
            outs=[nc.scalar.lower_ap(out)],
        )
    )


def _momentum_scalars():
    """a_j = fp16(1+gamma_j) (exact in fp16), b_j = 1 - a_j (exact fp32), so
    the net z-coefficient perturbation cancels; only gamma*(z - z_old) sees
    the ~2e-4 rounding, attenuated by |z - z_old|."""
    ts = [1.0]
    for _ in range(MAX_ITER + 2):
        ts.append((1.0 + np.sqrt(1.0 + 4.0 * ts[-1] ** 2)) / 2.0)
    al, bl = [], []
    for j in range(MAX_ITER):
        gam = 0.0 if j < 2 else (ts[j - 1] - 1.0) / ts[j]
        a_hat = float(np.float16(1.0 + gam))
        al.append(a_hat)
        bl.append(float(1.0 - a_hat))
    return al, bl


def build_nc():
    nc = bacc.Bacc(None)
    IDN_d = nc.declare_dram_parameter("IDN", [P, P], F32R, isOutput=False)
    R20_d = nc.declare_dram_parameter("R20", [P, 2 * B], FP16, isOutput=False)
    W2_d = nc.declare_dram_parameter("W2", [P, NCH, P], FP16, isOutput=False)
    X4_d = nc.declare_dram_parameter("X4", [P, 2 * B], F32, isOutput=False)
    W1_d = nc.declare_dram_parameter("W1", [P, NCH, P], F32R, isOutput=False)
    mag_d = nc.declare_dram_parameter("magT", [P, NCH, B], FP16, isOutput=True)

    alphas, betas = _momentum_scalars()

    with tile.TileContext(nc) as tc, ExitStack() as ctx:
        state = ctx.enter_context(tc.tile_pool(name="state", bufs=1))
        temps = ctx.enter_context(tc.tile_pool(name="temps", bufs=3))
        small = ctx.enter_context(tc.tile_pool(name="small", bufs=2))
        psum_u = ctx.enter_context(tc.tile_pool(name="psum_u", bufs=3, space="PSUM"))
        psum_p1 = ctx.enter_context(tc.tile_pool(name="psum_p1", bufs=2, space="PSUM"))

        # ---- persistent SBUF tensors
        IDN = state.tile([P, P], F32R, tag="IDN")
        R20 = state.tile([P, 2 * B], FP16, tag="R20")
        W2 = state.tile([P, NCH, P], FP16, tag="W2")
        X4 = state.tile([P, 2 * B], F32, tag="X4")
        W1 = state.tile([P, NCH, P], F32R, tag="W1")
        zA = state.tile([P, NCH, 2 * B], F32R, tag="zA")
        zB = state.tile([P, NCH, 2 * B], F32R, tag="zB")
        aI = state.tile([P, MAX_ITER, P], F32R, tag="aI")
        magT = state.tile([P, NCH, B], FP16, tag="magT")
        one_col = state.tile([P, 1], F32, tag="oc")
        eps_col = state.tile([P, 1], F32, tag="ec")
        zero_col = state.tile([P, 1], F32, tag="zc")

        nc.sync.dma_start(IDN[:], IDN_d[:])
        nc.sync.dma_start(R20[:], R20_d[:])
        nc.sync.dma_start(W2[:], W2_d[:])
        nc.sync.dma_start(X4[:], X4_d[:])
        nc.sync.dma_start(W1[:], W1_d[:])

        nc.vector.memset(one_col[:], 1.0)
        nc.vector.memset(eps_col[:], 1e-30)
        nc.vector.memset(zero_col[:], 0.0)
        # scaled identities for all iterations, built once upfront
        for j in range(2, MAX_ITER):
            nc.vector.tensor_scalar_mul(aI[:, j, :], IDN[:], alphas[j])

        zbuf = [zA, zB]
        r2cat = None      # fp16 residual stream for the current iteration
        qold = None       # b*P1_old - X4 for the upcoming boundary
        p1_prev = None

        for j in range(MAX_ITER):
            a, b = alphas[j], betas[j]
            last = j == MAX_ITER - 1
            z_cur = zbuf[j % 2]
            z_new = zbuf[(j + 1) % 2]   # holds z_{j-1} until overwritten

            p1_ps = None
            if not last:
                p1_ps = psum_p1.tile([P, 2 * B], F32, tag="P1")

            aIj = IDN[:] if j == 1 else (None if j == 0 else aI[:, j, :])

            u_tiles = []
            for g in range(NGRP):
                u_ps = psum_u.tile([P, GRP, 2 * B], F32, tag="u")
                # momentum a-term identities first (no r2cat dependency:
                # fills the iteration-boundary bubble)
                if j > 0:
                    for pi in range(GRP // 2):
                        c2 = GRP * g + 2 * pi
                        out_sl = u_ps[:, 2 * pi:2 * pi + 2, :].rearrange(
                            "p c n -> p (c n)")
                        nc.tensor.matmul(
                            out_sl, aIj,
                            z_cur[:, c2:c2 + 2, :].rearrange("p c n -> p (c n)"),
                            start=True, stop=False, skip_group_check=True,
                        )
                # gradient matmuls (need r2cat)
                rstream = R20 if j == 0 else r2cat
                for ci in range(GRP):
                    c = GRP * g + ci
                    nc.tensor.matmul(
                        u_ps[:, ci, :], W2[:, c, :], rstream[:],
                        start=(j == 0), stop=(j == 0 or ci == GRP - 1),
                        skip_group_check=True,
                    )
                u_tiles.append(u_ps)

                # A-chain of z_{j+1}, ALAG groups behind the group loop
                if not last and g >= ALAG:
                    ga = g - ALAG
                    for ci in range(GRP):
                        c = GRP * ga + ci
                        nc.tensor.matmul(
                            p1_ps[:], W1[:, c, :], z_new[:, c, :],
                            start=(c == 0), stop=(c == NCH - 1),
                            skip_group_check=True,
                        )

                # ---- elementwise chain for group g
                if j >= 2:
                    u = temps.tile([P, GRP, 2 * B], F32, tag="u_sb")
                    nc.vector.scalar_tensor_tensor(
                        u[:], z_new[:, GRP * g:GRP * (g + 1), :], b, u_ps[:],
                        ALU.mult, ALU.add,
                    )
                else:
                    u = u_ps
                t12 = temps.tile([P, GRP, 2 * B], F32, tag="t12")
                if j >= 2:
                    nc.vector.tensor_tensor(
                        t12[:, 0:2, :], u[:, 0:2, :], u[:, 0:2, :], ALU.mult)
                    nc.gpsimd.tensor_tensor(
                        t12[:, 2:4, :], u[:, 2:4, :], u[:, 2:4, :], ALU.mult)
                else:
                    # u is PSUM here; dual-PSUM-operand TT is not allowed, so
                    # square on the scalar engine instead
                    nc.scalar.activation(
                        t12[:], u[:], ACTF.Square, bias=zero_col[:])
                m2 = temps.tile([P, GRP, B], F32, tag="m2")
                nc.gpsimd.tensor_tensor(
                    m2[:], t12[:, :, 0:B], t12[:, :, B:2 * B], ALU.add)
                rsq = temps.tile([P, GRP, B], F32, tag="rsq")
                _activation_raw(nc, rsq[:], m2[:], ACTF.Rsqrt, bias=eps_col[:])
                s = temps.tile([P, GRP, B], F32, tag="srelu")
                nc.scalar.activation(
                    s[:], rsq[:], ACTF.Relu, bias=one_col[:], scale=-float(THR))

                if not last:
                    z_sl = z_new[:, GRP * g:GRP * (g + 1), :]
                    z_view = z_sl.rearrange("p c (t b) -> p c t b", t=2)
                    u_view = u[:].rearrange("p c (t b) -> p c t b", t=2)
                    s_b = s[:, :, None, :].to_broadcast([P, GRP, 2, B])
                    nc.vector.tensor_tensor(z_view, u_view, s_b, ALU.mult)
                else:
                    mag = temps.tile([P, GRP, B], F32, tag="mag")
                    nc.scalar.activation(mag[:], m2[:], ACTF.Sqrt, bias=eps_col[:])
                    nc.vector.tensor_tensor(
                        magT[:, GRP * g:GRP * (g + 1), :], mag[:], s[:], ALU.mult)
                    nc.sync.dma_start(
                        mag_d[:, GRP * g:GRP * (g + 1), :],
                        magT[:, GRP * g:GRP * (g + 1), :])

            if last:
                break

            # flush trailing A-chain groups
            for ga in range(NGRP - ALAG, NGRP):
                for ci in range(GRP):
                    c = GRP * ga + ci
                    nc.tensor.matmul(
                        p1_ps[:], W1[:, c, :], z_new[:, c, :],
                        start=(c == 0), stop=(c == NCH - 1),
                        skip_group_check=True,
                    )

            # ---- iteration boundary: build fp16 residual stream for j+1.
            # P1 quadrants: rows 0:T = [Dr w_r | Dr w_i], rows T:P =
            # [Di w_r | Di w_i]; X4 top rows subtract xr|xi. True residual:
            #   r_re = Rq[t, b] - Rq[T+t, B+b],  r_im = Rq[t, B+b] + Rq[T+t, b]
            # These folds cross partitions, so the bottom quadrants (and the
            # assembled swap-half) are realigned with SBUF->SBUF DMAs.
            a_n = alphas[j + 1]
            if j == 0:
                # qold for iter 1 is -X4 (P1_0 = 0, b_1 = 0)
                qold = small.tile([P, 2 * B], F32, tag="qold")
                nc.gpsimd.tensor_scalar_mul(qold[:], X4[:], -1.0)
            rq = small.tile([P, 2 * B], F32, tag="rq")
            nc.vector.scalar_tensor_tensor(
                rq[:], p1_ps[:], a_n, qold[:], ALU.mult, ALU.add)
            qbot = small.tile([T, 2 * B], F32, tag="qbot")
            nc.sync.dma_start(qbot[:], rq[T:P, :])
            r2cat = small.tile([P, 2 * B], FP16, tag="r2cat")
            rns = small.tile([T, 2 * B], FP16, tag="rns")
            # r_re into r2cat top-left; -r_re into rns right (operand swap)
            nc.gpsimd.tensor_tensor(
                r2cat[0:T, 0:B], rq[0:T, 0:B], qbot[:, B:2 * B], ALU.subtract)
            nc.gpsimd.tensor_tensor(
                rns[:, B:2 * B], qbot[:, B:2 * B], rq[0:T, 0:B], ALU.subtract)
            # r_im into r2cat top-right and rns left
            nc.vector.tensor_tensor(
                r2cat[0:T, B:2 * B], rq[0:T, B:2 * B], qbot[:, 0:B], ALU.add)
            nc.vector.tensor_tensor(
                rns[:, 0:B], rq[0:T, B:2 * B], qbot[:, 0:B], ALU.add)
            nc.sync.dma_start(r2cat[T:P, :], rns[:])
            if j + 2 < MAX_ITER:
                qold = small.tile([P, 2 * B], F32, tag="qold")
                nc.vector.scalar_tensor_tensor(
                    qold[:], p1_ps[:], betas[j + 2], X4[:],
                    ALU.mult, ALU.subtract)

    nc.finalize()
    return nc


def prep_host_inputs(x, D):
    """Builds per-core input maps from the full inputs."""
    Dr = np.ascontiguousarray(D.real).astype(np.float32)
    Di = np.ascontiguousarray(D.imag).astype(np.float32)
    W1c = np.concatenate(
        [Dr.T.reshape(NCH, P, T), Di.T.reshape(NCH, P, T)], axis=2
    )
    W1 = np.ascontiguousarray(W1c.transpose(1, 0, 2)).astype(np.float32)
    W2 = np.ascontiguousarray(
        np.concatenate([-STEP * Dr, -STEP * Di], axis=0).reshape(P, NCH, P)
    ).astype(np.float16)
    IDN = np.eye(P, dtype=np.float32)

    in_maps = []
    for i in range(NCORES):
        xs = x[i * B:(i + 1) * B]
        xr = np.ascontiguousarray(xs[:, 0].astype(np.float32).T)  # [T, B]
        xi = np.ascontiguousarray(xs[:, 1].astype(np.float32).T)
        X4 = np.zeros((P, 2 * B), dtype=np.float32)
        X4[0:T, 0:B] = xr
        X4[0:T, B:] = xi
        R20 = np.zeros((P, 2 * B), dtype=np.float16)
        R20[0:T, 0:B] = -xr
        R20[0:T, B:] = -xi
        R20[T:P, 0:B] = -xi
        R20[T:P, B:] = xr
        in_maps.append({
            "IDN": IDN, "R20": R20, "W2": W2, "X4": X4, "W1": W1,
        })
    return in_maps


def gather_output(results):
    outs = []
    for i in range(NCORES):
        magT = results[i]["magT"].reshape(P, NCH, B).astype(np.float32)
        outs.append(np.ascontiguousarray(magT.transpose(2, 1, 0)).reshape(B, F))
    mag_all = np.concatenate(outs, axis=0)
    return (mag_all / mag_all.max()).astype(np.float32)


_NC_CACHE = {}


def get_nc():
    if "nc" not in _NC_CACHE:
        _NC_CACHE["nc"] = build_nc()
    return _NC_CACHE["nc"]


def kernel(x, D):
    x = np.asarray(x)
    D = np.asarray(D)
    nc = get_nc()
    in_maps = prep_host_inputs(x, D)
    res = run_bass_kernel_spmd(nc, in_maps, list(range(NCORES)))
    return gather_output(res.results)


if __name__ == "__main__":
    import reference as ref
    inputs = ref.setup_inputs()
    out = kernel(**{k: np.asarray(v) for k, v in inputs.items()})
    print("kernel output", out.shape, out.dtype)


# revision 13
# speedup vs baseline: 1.2097x; 1.2097x over previous
"""Trainium2 Bass kernel for FISTA sparse coding (nn_FISTA_7550552506950).

Strategy (data-parallel over batch, 8 cores x 128 rows), v3:
- State z kept TRANSPOSED [F=4096, B=128] on-chip as float32r (~13-bit
  effective due to f32r write rounding - verified within tolerance), split
  into 32 f-chunks of [128, 256] (real|imag column halves).
- True-residual gradient: the tiny complex residual r = Dw - x is built once
  per iteration and streamed as the fp16 concatenation
  [r_re|r_im ; r_im|-r_re], so ONE fp16 matmul per chunk produces both
  gradient halves (fp16 safe: r is small, cancellation already done in fp32).
  The cross-partition quadrant folds use PE permutation matmuls (a [128->64]
  swap for the bottom quadrants, two [64->64] moves for the ns-half) instead
  of SBUF DMAs, keeping the boundary latency short and the PE streaming.
- A-chain (P1 = D @ z): one f32r matmul per chunk streaming z directly; the
  momentum combine A(w) = a*A(z) + b*A(z_old) happens on the tiny P1 tile
  via qold = b*P1_old - X4 (precomputed off the critical path).
- Momentum a-term (a*z) enters PSUM via f32r scaled-identity matmuls;
  b-term (b*z_old) is added by a fp32-exact DVE scalar_tensor_tensor.
  a = fp16-rounded (exact), b = 1-a fp32 => coefficient rounding cancels.
- Soft-threshold: t12 = (k*u)^2 in fp16 (k=1024 keeps the squares in fp16
  normal range; ACT chunks 0-1, GPSIMD STT chunks 2-3), m2 fp16 (GPSIMD),
  rsq = Rsqrt(m2/k^2) (ACT raw), s = relu(1-thr*rsq) (ACT, fp32),
  z = u*s as two contiguous TTs (DVE). Final |z| = sqrt(m2)/k * s as fp16.
- Global max normalization on host during the gather (tiny).
"""

import numpy as np
from contextlib import ExitStack

import concourse.bass as bass
import concourse.mybir as mybir
import concourse.tile as tile
from concourse import bacc
from concourse.bass_utils import run_bass_kernel_spmd

F32 = mybir.dt.float32
F32R = mybir.dt.float32r
BF16 = mybir.dt.bfloat16
FP16 = mybir.dt.float16
ALU = mybir.AluOpType
ACTF = mybir.ActivationFunctionType

P = 128          # partitions / f-chunk size
F = 4096         # dictionary size
T = 64           # signal dim
NCH = F // P     # 32 chunks
B = 128          # batch rows per core
NCORES = 8
MAX_ITER = 25
STEP = np.float32(1.0 / F)
THR = np.float32(0.5) * STEP
GRP = 4          # chunks per elementwise group
NGRP = NCH // GRP
ALAG = 2         # groups of delay before A-chain streams fresh z
KAPPA = 1024.0   # fp16 square pre-scale


def _activation_raw(nc, out, in_, func, bias, scale=1.0):
    """nc.scalar.activation minus the Rsqrt accuracy guard.

    Safe here: rsqrt feeds only the soft-threshold scale, where its error is
    attenuated by thr/mag; the final output magnitude uses Sqrt instead.
    """
    inputs = [nc.scalar.lower_ap(in_)]
    for arg in (bias, scale, 0.0):
        if isinstance(arg, float):
            inputs.append(mybir.ImmediateValue(dtype=F32, value=arg))
        else:
            inputs.append(nc.scalar.lower_ap(arg))
    return nc.scalar.add_instruction(
        mybir.InstActivation(
            name=nc.get_next_instruction_name(),
            func=func,
            ins=inputs,
            outs=[nc.scalar.lower_ap(out)],
        )
    )


def _momentum_scalars():
    """a_j = fp16(1+gamma_j) (exact in fp16), b_j = 1 - a_j (exact fp32)."""
    ts = [1.0]
    for _ in range(MAX_ITER + 2):
        ts.append((1.0 + np.sqrt(1.0 + 4.0 * ts[-1] ** 2)) / 2.0)
    al, bl = [], []
    for j in range(MAX_ITER):
        gam = 0.0 if j < 2 else (ts[j - 1] - 1.0) / ts[j]
        a_hat = float(np.float16(1.0 + gam))
        al.append(a_hat)
        bl.append(float(1.0 - a_hat))
    return al, bl


def build_nc():
    nc = bacc.Bacc(None)
    IDN_d = nc.declare_dram_parameter("IDN", [P, P], F32R, isOutput=False)
    R20_d = nc.declare_dram_parameter("R20", [P, 2 * B], FP16, isOutput=False)
    W2_d = nc.declare_dram_parameter("W2", [P, NCH, P], FP16, isOutput=False)
    X4_d = nc.declare_dram_parameter("X4", [P, 2 * B], F32, isOutput=False)
    SSW_d = nc.declare_dram_parameter("SSW", [P, T], F32R, isOutput=False)
    I64P_d = nc.declare_dram_parameter("I64P", [T, T], FP16, isOutput=False)
    I64N_d = nc.declare_dram_parameter("I64N", [T, T], FP16, isOutput=False)
    W1_d = nc.declare_dram_parameter("W1", [P, NCH, P], F32R, isOutput=False)
    mag_d = nc.declare_dram_parameter("magT", [P, NCH, B], FP16, isOutput=True)

    alphas, betas = _momentum_scalars()

    with tile.TileContext(nc) as tc, ExitStack() as ctx:
        state = ctx.enter_context(tc.tile_pool(name="state", bufs=1))
        temps = ctx.enter_context(tc.tile_pool(name="temps", bufs=3))
        small = ctx.enter_context(tc.tile_pool(name="small", bufs=2))
        psum_u = ctx.enter_context(tc.tile_pool(name="psum_u", bufs=3, space="PSUM"))
        psum_p1 = ctx.enter_context(tc.tile_pool(name="psum_p1", bufs=1, space="PSUM"))
        psum_b = ctx.enter_context(tc.tile_pool(name="psum_b", bufs=1, space="PSUM"))

        # ---- persistent SBUF tensors
        IDN = state.tile([P, P], F32R, tag="IDN")
        R20 = state.tile([P, 2 * B], FP16, tag="R20")
        W2 = state.tile([P, NCH, P], FP16, tag="W2")
        X4 = state.tile([P, 2 * B], F32, tag="X4")
        SSW = state.tile([P, T], F32R, tag="SSW")
        I64P = state.tile([T, T], FP16, tag="I64P")
        I64N = state.tile([T, T], FP16, tag="I64N")
        W1 = state.tile([P, NCH, P], F32R, tag="W1")
        zA = state.tile([P, NCH, 2 * B], F32R, tag="zA")
        zB = state.tile([P, NCH, 2 * B], F32R, tag="zB")
        aI = state.tile([P, MAX_ITER, P], F32R, tag="aI")
        magT = state.tile([P, NCH, B], FP16, tag="magT")
        one_col = state.tile([P, 1], F32, tag="oc")
        eps_col = state.tile([P, 1], F32, tag="ec")
        zero_col = state.tile([P, 1], F32, tag="zc")

        nc.sync.dma_start(IDN[:], IDN_d[:])
        nc.sync.dma_start(R20[:], R20_d[:])
        nc.sync.dma_start(W2[:], W2_d[:])
        nc.sync.dma_start(X4[:], X4_d[:])
        nc.sync.dma_start(SSW[:], SSW_d[:])
        nc.sync.dma_start(I64P[:], I64P_d[:])
        nc.sync.dma_start(I64N[:], I64N_d[:])
        nc.sync.dma_start(W1[:], W1_d[:])

        nc.vector.memset(one_col[:], 1.0 / KAPPA)
        nc.vector.memset(eps_col[:], 1e-30)
        nc.vector.memset(zero_col[:], 0.0)
        # scaled identities for all iterations, built once upfront
        for j in range(1, MAX_ITER):
            nc.vector.tensor_scalar_mul(aI[:, j, :], IDN[:], KAPPA * alphas[j])

        zbuf = [zA, zB]
        r2cat = None      # fp16 residual stream for the current iteration
        qold = None       # b*P1_old - X4 for the upcoming boundary

        def emit_mom_a(j, g, u_ps):
            aIj = aI[:, j, :]
            for pi in range(GRP // 2):
                c2 = GRP * g + 2 * pi
                out_sl = u_ps[:, 2 * pi:2 * pi + 2, :].rearrange(
                    "p c n -> p (c n)")
                nc.tensor.matmul(
                    out_sl, aIj,
                    zbuf[j % 2][:, c2:c2 + 2, :].rearrange("p c n -> p (c n)"),
                    start=True, stop=False, skip_group_check=True,
                )

        def emit_achain(j, ga, p1_ps):
            z_new = zbuf[(j + 1) % 2]
            for ci in range(GRP):
                c = GRP * ga + ci
                nc.tensor.matmul(
                    p1_ps[:], W1[:, c, :], z_new[:, c, :],
                    start=(c == 0), stop=(c == NCH - 1),
                    skip_group_check=True,
                )

        u_pending = None  # u_ps tile allocated early for next iter's group 0

        for j in range(MAX_ITER):
            b = betas[j]
            last = j == MAX_ITER - 1
            z_new = zbuf[(j + 1) % 2]   # holds z_{j-1} until overwritten

            p1_ps = None
            if not last:
                p1_ps = psum_p1.tile([P, 2 * B], F32, tag="P1")

            for g in range(NGRP):
                if g == 0 and u_pending is not None:
                    u_ps = u_pending
                    u_pending = None
                else:
                    u_ps = psum_u.tile([P, GRP, 2 * B], F32, tag="u")
                    if j > 0:
                        emit_mom_a(j, g, u_ps)
                # gradient matmuls (need r2cat)
                rstream = R20 if j == 0 else r2cat
                for ci in range(GRP):
                    c = GRP * g + ci
                    nc.tensor.matmul(
                        u_ps[:, ci, :], W2[:, c, :], rstream[:],
                        start=(j == 0), stop=(j == 0 or ci == GRP - 1),
                        skip_group_check=True,
                    )

                # A-chain of z_{j+1}, ALAG groups behind the group loop
                if not last and g >= ALAG:
                    emit_achain(j, g - ALAG, p1_ps)

                # ---- elementwise chain for group g
                if j >= 2:
                    u = temps.tile([P, GRP, 2 * B], F32, tag="u_sb")
                    nc.vector.scalar_tensor_tensor(
                        u[:], z_new[:, GRP * g:GRP * (g + 1), :], KAPPA * b,
                        u_ps[:], ALU.mult, ALU.add,
                    )
                else:
                    u = u_ps
                t12 = temps.tile([P, GRP, 2 * B], FP16, tag="t12")
                if j >= 2:
                    nc.scalar.activation(
                        t12[:, 0:2, :], u[:, 0:2, :], ACTF.Square,
                        bias=zero_col[:])
                    nc.gpsimd.tensor_tensor(
                        t12[:, 2:4, :], u[:, 2:4, :], u[:, 2:4, :], ALU.mult)
                else:
                    # u is PSUM here (GPSIMD cannot read PSUM)
                    nc.scalar.activation(
                        t12[:], u[:], ACTF.Square, bias=zero_col[:])
                m2 = temps.tile([P, GRP, B], FP16, tag="m2")
                nc.gpsimd.tensor_tensor(
                    m2[:], t12[:, :, 0:B], t12[:, :, B:2 * B], ALU.add)
                rsq = temps.tile([P, GRP, B], F32, tag="rsq")
                _activation_raw(nc, rsq[:], m2[:], ACTF.Rsqrt, bias=eps_col[:])
                s = temps.tile([P, GRP, B], F32, tag="srelu")
                nc.scalar.activation(
                    s[:], rsq[:], ACTF.Relu, bias=one_col[:], scale=-float(THR))

                if not last:
                    z_sl = z_new[:, GRP * g:GRP * (g + 1), :]
                    nc.vector.tensor_tensor(
                        z_sl[:, :, 0:B], u[:, :, 0:B], s[:], ALU.mult)
                    nc.vector.tensor_tensor(
                        z_sl[:, :, B:2 * B], u[:, :, B:2 * B], s[:], ALU.mult)
                else:
                    mag = temps.tile([P, GRP, B], F32, tag="mag")
                    nc.scalar.activation(
                        mag[:], m2[:], ACTF.Sqrt, bias=eps_col[:])
                    nc.vector.tensor_tensor(
                        magT[:, GRP * g:GRP * (g + 1), :], mag[:], s[:],
                        ALU.mult)
                    nc.sync.dma_start(
                        mag_d[:, GRP * g:GRP * (g + 1), :],
                        magT[:, GRP * g:GRP * (g + 1), :])

            if last:
                break

            # flush trailing A-chain groups
            for ga in range(NGRP - ALAG, NGRP):
                emit_achain(j, ga, p1_ps)

            # ---- iteration boundary: build the fp16 residual stream for
            # j+1. P1 quadrants: rows 0:T = [Dr w_r | Dr w_i], rows T:P =
            # [Di w_r | Di w_i]; X4 top rows carry xr|xi. True residual:
            #   r_re = Rq[t, b] - Rq[T+t, B+b],  r_im = Rq[t, B+b] + Rq[T+t, b]
            # Partition realignment runs on the PE: SSW swaps the bottom
            # quadrant rows up; I64P/I64N build the [r_im | -r_re] bottom half.
            a_n = alphas[j + 1]
            if j == 0:
                qold = small.tile([P, 2 * B], F32, tag="qold")
                nc.gpsimd.tensor_scalar_mul(qold[:], X4[:], -1.0)
            rq = small.tile([P, 2 * B], F32R, tag="rq")
            nc.vector.scalar_tensor_tensor(
                rq[:], p1_ps[:], a_n, qold[:], ALU.mult, ALU.add)
            qb = psum_b.tile([P, 2, 2 * B], F32, tag="qb")
            qsw = qb[:, 0, :]
            nc.tensor.matmul(qsw[0:T, :], SSW[:], rq[:],
                             start=True, stop=True, skip_group_check=True)
            # next iteration's first momentum group fills the fold latency
            u_pending = psum_u.tile([P, GRP, 2 * B], F32, tag="u")
            emit_mom_a(j + 1, 0, u_pending)
            r2cat = small.tile([P, 2 * B], FP16, tag="r2cat")
            nc.vector.tensor_tensor(
                r2cat[0:T, 0:B], rq[0:T, 0:B], qsw[0:T, B:2 * B], ALU.subtract)
            nc.vector.tensor_tensor(
                r2cat[0:T, B:2 * B], rq[0:T, B:2 * B], qsw[0:T, 0:B], ALU.add)
            bot = qb[:, 1, :]
            nc.tensor.matmul(bot[T:P, 0:B], I64P[:], r2cat[0:T, B:2 * B],
                             start=True, stop=False, skip_group_check=True)
            nc.tensor.matmul(bot[T:P, B:2 * B], I64N[:], r2cat[0:T, 0:B],
                             start=False, stop=True, skip_group_check=True)
            nc.scalar.copy(r2cat[T:P, :], bot[T:P, :])
            if j + 2 < MAX_ITER:
                qold = small.tile([P, 2 * B], F32, tag="qold")
                nc.vector.scalar_tensor_tensor(
                    qold[:], p1_ps[:], betas[j + 2], X4[:],
                    ALU.mult, ALU.subtract)

    nc.finalize()
    return nc


def prep_host_inputs(x, D):
    """Builds per-core input maps from the full inputs."""
    Dr = np.ascontiguousarray(D.real).astype(np.float32)
    Di = np.ascontiguousarray(D.imag).astype(np.float32)
    W1c = np.concatenate(
        [Dr.T.reshape(NCH, P, T), Di.T.reshape(NCH, P, T)], axis=2
    )
    W1 = np.ascontiguousarray(W1c.transpose(1, 0, 2)).astype(np.float32)
    W2 = np.ascontiguousarray(
        KAPPA * np.concatenate([-STEP * Dr, -STEP * Di], axis=0).reshape(P, NCH, P)
    ).astype(np.float16)
    IDN = np.eye(P, dtype=np.float32)
    SSW = np.zeros((P, T), dtype=np.float32)
    for m in range(T):
        SSW[T + m, m] = 1.0
    I64P = np.eye(T, dtype=np.float16)
    I64N = -np.eye(T, dtype=np.float16)

    in_maps = []
    for i in range(NCORES):
        xs = x[i * B:(i + 1) * B]
        xr = np.ascontiguousarray(xs[:, 0].astype(np.float32).T)  # [T, B]
        xi = np.ascontiguousarray(xs[:, 1].astype(np.float32).T)
        X4 = np.zeros((P, 2 * B), dtype=np.float32)
        X4[0:T, 0:B] = xr
        X4[0:T, B:] = xi
        R20 = np.zeros((P, 2 * B), dtype=np.float16)
        R20[0:T, 0:B] = -xr
        R20[0:T, B:] = -xi
        R20[T:P, 0:B] = -xi
        R20[T:P, B:] = xr
        in_maps.append({
            "IDN": IDN, "R20": R20, "W2": W2, "X4": X4, "W1": W1,
            "SSW": SSW, "I64P": I64P, "I64N": I64N,
        })
    return in_maps


def gather_output(results):
    outs = []
    for i in range(NCORES):
        magT = results[i]["magT"].reshape(P, NCH, B).astype(np.float32)
        outs.append(np.ascontiguousarray(magT.transpose(2, 1, 0)).reshape(B, F))
    mag_all = np.concatenate(outs, axis=0)
    return (mag_all / mag_all.max()).astype(np.float32)


_NC_CACHE = {}


def get_nc():
    if "nc" not in _NC_CACHE:
        _NC_CACHE["nc"] = build_nc()
    return _NC_CACHE["nc"]


def kernel(x, D):
    x = np.asarray(x)
    D = np.asarray(D)
    nc = get_nc()
    in_maps = prep_host_inputs(x, D)
    res = run_bass_kernel_spmd(nc, in_maps, list(range(NCORES)))
    return gather_output(res.results)


if __name__ == "__main__":
    import reference as ref
    inputs = ref.setup_inputs()
    out = kernel(**{k: np.asarray(v) for k, v in inputs.items()})
    print("kernel output", out.shape, out.dtype)
